# revision 9
# baseline (speedup 1.0000x reference)
"""AttentionBlock (GroupNorm -> QKV -> 4-head attention over S=4096 -> out-proj
-> residual) on 8 TRN2 NeuronCores.

Sharding: data-parallel over (batch, query-half). Core c handles batch c//2 and
query rows [half*2048, (half+1)*2048) where half = c % 2. All cores run the
same SPMD graph; the query-half split is achieved by rolling the spatial axis
of x per core, which leaves GroupNorm stats and attention (softmax over all
keys) invariant. No collectives needed; host concatenates per-core outputs.

Kernel structure per core (everything stays on-chip after one 4MB x load):
  1. GroupNorm via bn_stats + two tiny cross-partition matmul reductions.
  2. Q^T/K^T computed directly transposed ([d=64 part, s free]) so the score
     matmul needs no transposes; V computed in [s part, d free] with an
     appended ones-column.
  3. Per (head, 1024-query chunk): loop over 32 key blocks of 128:
     scores^T block [sk=128, sq=1024] on PE -> exp(0.125*x) on ScalarE (no
     max-subtraction: scores are provably O(+-8) here) -> PV matmul
     accumulating [65, 1024] in PSUM, where row 64 accumulates the softmax
     denominator (ones column of V).
  4. Normalize by the denominator (DMA partition-broadcast of the reciprocal
     row), out-projection accumulated over heads in PSUM, + bias + residual.
"""

import os
import numpy as np

import concourse.bass as bass
import concourse.tile as tile
from concourse import bacc, mybir
from concourse.bass_utils import run_bass_kernel_spmd

B, C, H, W = 4, 256, 64, 64
S = H * W            # 4096 spatial positions
NG = 32              # groupnorm groups
GS = C // NG         # 8 channels / group
NH, DK = 4, 64       # heads, head dim
SQ = S // 2          # queries per core (2048)
P = 128              # partitions
KC = C // P          # channel chunks (2)
EPS = 1e-5
SCALE = DK ** -0.5   # 0.125
N_CORES = 8

F32 = mybir.dt.float32
BF16 = mybir.dt.bfloat16
AL = mybir.AluOpType


def _build_nc():
    nc = bacc.Bacc("TRN2", target_bir_lowering=False, debug=False)

    d_x = nc.dram_tensor("x", [C, S], F32, kind="ExternalInput")
    d_gamma = nc.dram_tensor("gamma", [C], F32, kind="ExternalInput")
    d_beta = nc.dram_tensor("beta", [C], F32, kind="ExternalInput")
    d_wqkv = nc.dram_tensor("w_qkv", [C, 3 * NH * DK], F32, kind="ExternalInput")
    d_bqkv = nc.dram_tensor("b_qkv", [3 * NH * DK], F32, kind="ExternalInput")
    d_wout = nc.dram_tensor("w_out", [NH * DK, C], F32, kind="ExternalInput")
    d_bout = nc.dram_tensor("b_out", [C], F32, kind="ExternalInput")
    # group-average / group-broadcast one-hot matrices (host-precomputed)
    d_c2g = nc.dram_tensor("c2g", [C, NG], F32, kind="ExternalInput")
    d_g2c = nc.dram_tensor("g2c", [NG, C], F32, kind="ExternalInput")
    d_out = nc.dram_tensor("out", [C, SQ], F32, kind="ExternalOutput")

    dbg = {}
    if os.environ.get("KDBG"):
        dbg["h"] = nc.dram_tensor("dbg_h", [P, KC, S], BF16, kind="ExternalOutput")
        dbg["qt"] = nc.dram_tensor("dbg_qt", [DK, NH, SQ], BF16, kind="ExternalOutput")
        dbg["kt"] = nc.dram_tensor("dbg_kt", [DK, NH, S], BF16, kind="ExternalOutput")
        dbg["v"] = nc.dram_tensor("dbg_v", [P, S // P, NH, DK + 1], BF16,
                                  kind="ExternalOutput")
        dbg["at"] = nc.dram_tensor("dbg_at", [DK, NH, SQ], BF16, kind="ExternalOutput")

    with tile.TileContext(nc) as tc:
        _body(tc, d_x, d_gamma, d_beta, d_wqkv, d_bqkv, d_wout, d_bout,
              d_c2g, d_g2c, d_out, dbg)

    nc.compile()
    return nc


def _body(tc, d_x, d_gamma, d_beta, d_wqkv, d_bqkv, d_wout, d_bout,
          d_c2g, d_g2c, d_out, dbg=None):
    dbg = dbg or {}
    nc = tc.nc
    from contextlib import ExitStack

    with ExitStack() as ctx:
        consts = ctx.enter_context(tc.tile_pool(name="consts", bufs=1))
        big = ctx.enter_context(tc.tile_pool(name="big", bufs=1))
        work = ctx.enter_context(tc.tile_pool(name="work", bufs=2))

        # ------------- loads -------------
        x_sb = big.tile([P, KC, S], F32, name="x_sb")
        nc.sync.dma_start(out=x_sb, in_=d_x.ap().rearrange("(k p) s -> p k s", p=P))

        c2g = consts.tile([P, KC, NG], F32, name="c2g_sb")
        nc.sync.dma_start(out=c2g, in_=d_c2g.ap().rearrange("(k p) g -> p k g", p=P))
        g2c = consts.tile([NG, C], F32, name="g2c_sb")
        nc.sync.dma_start(out=g2c, in_=d_g2c.ap())
        gam = consts.tile([P, KC], F32, name="gam_sb")
        nc.sync.dma_start(out=gam, in_=d_gamma.ap().rearrange("(k p) -> p k", p=P))
        bet = consts.tile([P, KC], F32, name="bet_sb")
        nc.sync.dma_start(out=bet, in_=d_beta.ap().rearrange("(k p) -> p k", p=P))
        bout = consts.tile([P, KC], F32, name="bout_sb")
        nc.sync.dma_start(out=bout, in_=d_bout.ap().rearrange("(k p) -> p k", p=P))

        wqkv_f = big.tile([P, KC, 3 * NH * DK], F32, name="wqkv_f")
        nc.sync.dma_start(out=wqkv_f,
                          in_=d_wqkv.ap().rearrange("(k p) n -> p k n", p=P))
        wqkv = big.tile([P, KC, 3 * NH * DK], BF16, name="wqkv_sb")
        nc.vector.tensor_copy(out=wqkv, in_=wqkv_f)

        wout_f = big.tile([DK, NH, C], F32, name="wout_f")
        nc.sync.dma_start(out=wout_f,
                          in_=d_wout.ap().rearrange("(h d) c -> d h c", h=NH))
        wout = big.tile([DK, NH, C], BF16, name="wout_sb")
        nc.vector.tensor_copy(out=wout, in_=wout_f)

        # qkv bias: b_qkv layout is (head, {q,k,v}, d); load as [64, h*3+t]
        bqkv = consts.tile([DK, 3 * NH], F32, name="bqkv_sb")
        nc.sync.dma_start(
            out=bqkv,
            in_=d_bqkv.ap().rearrange("(x d) -> d x", d=DK))
        bq = bqkv.rearrange("d (h t) -> d t h", t=3)[:, 0, :]
        bk = bqkv.rearrange("d (h t) -> d t h", t=3)[:, 1, :]
        bv = bqkv.rearrange("d (h t) -> d t h", t=3)[:, 2, :]

        eps_t = consts.tile([NG, 1], F32, name="eps_t")
        nc.vector.memset(eps_t, EPS)

        # ------------- GroupNorm -------------
        h_sb = big.tile([P, KC, S], BF16, name="h_sb")
        with tc.tile_pool(name="psA", bufs=2, space="PSUM") as psA:
            gp = psA.tile([NG, 3], F32, tag="gp", bufs=1)
            for k in range(KC):
                st = work.tile([P, 8, 6], F32, tag="bnst")
                for sub in range(8):
                    nc.vector.bn_stats(out=st[:, sub, :],
                                       in_=x_sb[:, k, sub * 512:(sub + 1) * 512])
                mv = work.tile([P, 2], F32, tag="mv")
                nc.vector.bn_aggr(out=mv, in_=st)
                s3 = work.tile([P, 3], F32, tag="s3")
                nc.vector.tensor_copy(out=s3[:, 0:2], in_=mv)
                nc.vector.tensor_mul(out=s3[:, 2:3], in0=mv[:, 0:1], in1=mv[:, 0:1])
                # group stats: [32, 3] = (mean, E[var], E[mean^2]) averaged over
                # the 8 channels of each group (c2g holds 1/8)
                nc.tensor.matmul(out=gp, lhsT=c2g[:, k, :], rhs=s3,
                                 start=(k == 0), stop=(k == KC - 1))
            gsb = work.tile([NG, 3], F32, tag="gsb")
            nc.vector.tensor_copy(out=gsb, in_=gp)
            varg = work.tile([NG, 1], F32, tag="varg")
            nc.vector.tensor_add(out=varg, in0=gsb[:, 1:2], in1=gsb[:, 2:3])
            m2 = work.tile([NG, 1], F32, tag="m2")
            nc.vector.tensor_mul(out=m2, in0=gsb[:, 0:1], in1=gsb[:, 0:1])
            nc.vector.tensor_sub(out=varg, in0=varg, in1=m2)
            stdg = work.tile([NG, 1], F32, tag="stdg")
            nc.scalar.activation(out=stdg, in_=varg,
                                 func=mybir.ActivationFunctionType.Sqrt,
                                 bias=eps_t, scale=1.0)
            rstdg = work.tile([NG, 1], F32, tag="rstdg")
            nc.vector.reciprocal(out=rstdg, in_=stdg)
            gs2 = work.tile([NG, 2], F32, tag="gs2")
            nc.vector.tensor_copy(out=gs2[:, 0:1], in_=gsb[:, 0:1])
            nc.vector.tensor_copy(out=gs2[:, 1:2], in_=rstdg)

            for k in range(KC):
                cp = psA.tile([P, 2], F32, tag="cp")
                nc.tensor.matmul(out=cp, lhsT=g2c[:, k * P:(k + 1) * P], rhs=gs2,
                                 start=True, stop=True)
                scale_c = work.tile([P, 1], F32, tag="scale_c")
                nc.vector.tensor_mul(out=scale_c, in0=gam[:, k:k + 1],
                                     in1=cp[:, 1:2])
                tmpc = work.tile([P, 1], F32, tag="tmpc")
                nc.vector.tensor_mul(out=tmpc, in0=cp[:, 0:1], in1=scale_c)
                bias_c = work.tile([P, 1], F32, tag="bias_c")
                nc.vector.tensor_sub(out=bias_c, in0=bet[:, k:k + 1], in1=tmpc)
                nc.vector.tensor_scalar(out=h_sb[:, k, :], in0=x_sb[:, k, :],
                                        scalar1=scale_c, scalar2=bias_c,
                                        op0=AL.mult, op1=AL.add)

        if "h" in dbg:
            nc.sync.dma_start(out=dbg["h"].ap(), in_=h_sb)

        # ------------- QKV projections -------------
        QT = big.tile([DK, NH, SQ], BF16, name="QT")
        KT = big.tile([DK, NH, S], BF16, name="KT")
        V = big.tile([P, S // P, NH, DK + 1], BF16, name="V")
        nc.vector.memset(V[:, :, :, DK:DK + 1], 1.0)

        with tc.tile_pool(name="psB", bufs=3, space="PSUM") as psB:
            for h in range(NH):
                for dst, colbase, srcS, btile in (
                        (KT, h * 3 * DK + DK, S, bk),
                        (QT, h * 3 * DK, SQ, bq)):
                    for stn in range(srcS // 512):
                        pq = psB.tile([DK, 512], F32, tag="qk")
                        for k in range(KC):
                            nc.tensor.matmul(
                                out=pq,
                                lhsT=wqkv[:, k, colbase:colbase + DK],
                                rhs=h_sb[:, k, stn * 512:(stn + 1) * 512],
                                start=(k == 0), stop=(k == KC - 1))
                        nc.vector.tensor_scalar(
                            out=dst[:, h, stn * 512:(stn + 1) * 512],
                            in0=pq, scalar1=btile[:, h:h + 1], scalar2=None,
                            op0=AL.add)
            for sc in range(S // P):
                pv = psB.tile([P, NH * DK], F32, tag="v")
                for k in range(KC):
                    wv_view = wqkv[:, k, :].rearrange(
                        "p (h t d) -> p t h d", h=NH, t=3)[:, 2, :, :]
                    nc.tensor.matmul(out=pv,
                                     lhsT=h_sb[:, k, sc * P:(sc + 1) * P],
                                     rhs=wv_view,
                                     start=(k == 0), stop=(k == KC - 1))
                nc.vector.tensor_copy(
                    out=V[:, sc, :, 0:DK],
                    in_=pv.rearrange("p (h d) -> p h d", h=NH))

        if "qt" in dbg:
            nc.sync.dma_start(out=dbg["qt"].ap(), in_=QT)
            nc.sync.dma_start(out=dbg["kt"].ap(), in_=KT)
            nc.sync.dma_start(out=dbg["v"].ap(), in_=V)

        # ------------- attention -------------
        aT = big.tile([DK, NH, SQ], BF16, name="aT")
        CH = 1024  # query chunk
        with tc.tile_pool(name="psC", bufs=2, space="PSUM") as psC, \
                tc.tile_pool(name="ptp", bufs=3) as ptp, \
                tc.tile_pool(name="rbp", bufs=2) as rbp:
            for h in range(NH):
                for chi in range(SQ // CH):
                    q0 = chi * CH
                    pvacc = psC.tile([DK + 1, CH], F32, tag="pv")
                    for j in range(S // P):
                        sc_ps = psC.tile([P, CH], F32, tag="sc")
                        for hf in range(CH // 512):
                            nc.tensor.matmul(
                                out=sc_ps[:, hf * 512:(hf + 1) * 512],
                                lhsT=KT[:, h, j * P:(j + 1) * P],
                                rhs=QT[:, h, q0 + hf * 512:q0 + (hf + 1) * 512],
                                start=True, stop=True)
                        pT = ptp.tile([P, CH], BF16, tag="pt")
                        nc.scalar.activation(out=pT, in_=sc_ps,
                                             func=mybir.ActivationFunctionType.Exp,
                                             scale=SCALE)
                        for hf in range(CH // 512):
                            nc.tensor.matmul(
                                out=pvacc[:, hf * 512:(hf + 1) * 512],
                                lhsT=V[:, j, h, :],
                                rhs=pT[:, hf * 512:(hf + 1) * 512],
                                start=(j == 0), stop=(j == S // P - 1))
                    # normalize: aT = pvacc[0:64] * (1 / l) + b_v
                    rec = rbp.tile([1, CH], F32, tag="rec")
                    nc.vector.reciprocal(out=rec, in_=pvacc[DK:DK + 1, :])
                    rb = rbp.tile([DK, CH], F32, tag="rb")
                    nc.gpsimd.partition_broadcast(rb, rec)
                    nc.vector.tensor_mul(out=aT[:, h, q0:q0 + CH],
                                         in0=pvacc[0:DK, :], in1=rb)
                    nc.vector.tensor_scalar(out=aT[:, h, q0:q0 + CH],
                                            in0=aT[:, h, q0:q0 + CH],
                                            scalar1=bv[:, h:h + 1], scalar2=None,
                                            op0=AL.add)

        if "at" in dbg:
            nc.sync.dma_start(out=dbg["at"].ap(), in_=aT)

        # ------------- out projection + residual -------------
        out_view = d_out.ap().rearrange("(k p) s -> p k s", p=P)
        with tc.tile_pool(name="psD", bufs=2, space="PSUM") as psD, \
                tc.tile_pool(name="ost", bufs=3) as ost:
            for k in range(KC):
                for stn in range(SQ // 512):
                    po = psD.tile([P, 512], F32, tag="op")
                    for h in range(NH):
                        nc.tensor.matmul(out=po,
                                         lhsT=wout[:, h, k * P:(k + 1) * P],
                                         rhs=aT[:, h, stn * 512:(stn + 1) * 512],
                                         start=(h == 0), stop=(h == NH - 1))
                    ob = ost.tile([P, 512], F32, tag="ob")
                    nc.vector.tensor_scalar(out=ob, in0=po,
                                            scalar1=bout[:, k:k + 1], scalar2=None,
                                            op0=AL.add)
                    nc.vector.tensor_add(out=ob, in0=ob,
                                         in1=x_sb[:, k, stn * 512:(stn + 1) * 512])
                    nc.sync.dma_start(out=out_view[:, k, stn * 512:(stn + 1) * 512],
                                      in_=ob)


_NC_CACHE = {}


def _get_nc():
    if "nc" not in _NC_CACHE:
        _NC_CACHE["nc"] = _build_nc()
    return _NC_CACHE["nc"]


def _host_constants():
    c2g = np.zeros((C, NG), np.float32)
    c2g[np.arange(C), np.arange(C) // GS] = 1.0 / GS
    g2c = np.zeros((NG, C), np.float32)
    g2c[np.arange(C) // GS, np.arange(C)] = 1.0
    return c2g, g2c


def make_in_maps(x, gamma, beta, w_qkv, b_qkv, w_out, b_out):
    c2g, g2c = _host_constants()
    com = {
        "gamma": np.ascontiguousarray(np.asarray(gamma, np.float32)),
        "beta": np.ascontiguousarray(np.asarray(beta, np.float32)),
        "w_qkv": np.ascontiguousarray(np.asarray(w_qkv, np.float32)),
        "b_qkv": np.ascontiguousarray(np.asarray(b_qkv, np.float32)),
        "w_out": np.ascontiguousarray(np.asarray(w_out, np.float32)),
        "b_out": np.ascontiguousarray(np.asarray(b_out, np.float32)),
        "c2g": c2g, "g2c": g2c,
    }
    x = np.asarray(x, np.float32)
    in_maps = []
    for c in range(N_CORES):
        b, half = divmod(c, 2)
        xb = x[b].reshape(C, S)
        if half:
            xb = np.roll(xb, -half * SQ, axis=1)
        in_maps.append({"x": np.ascontiguousarray(xb), **com})
    return in_maps


def assemble(results):
    out = np.empty((B, C, S), np.float32)
    for c in range(N_CORES):
        b, half = divmod(c, 2)
        out[b, :, half * SQ:(half + 1) * SQ] = results[c]["out"]
    return out.reshape(B, C, H, W)


def kernel(x, gamma, beta, w_qkv, b_qkv, w_out, b_out):
    nc = _get_nc()
    in_maps = make_in_maps(x, gamma, beta, w_qkv, b_qkv, w_out, b_out)
    res = run_bass_kernel_spmd(nc, in_maps, list(range(N_CORES)))
    return assemble(res.results)


# revision 11
# speedup vs baseline: 1.0019x; 1.0019x over previous
"""AttentionBlock (GroupNorm -> QKV -> 4-head attention over S=4096 -> out-proj
-> residual) on 8 TRN2 NeuronCores.

Sharding: data-parallel over (batch, query-half). Core c handles batch c//2 and
query rows [half*2048, (half+1)*2048) where half = c % 2. All cores run the
same SPMD graph; the query-half split is achieved by rolling the spatial axis
of x per core, which leaves GroupNorm stats and attention (softmax over all
keys) invariant. No collectives needed; host concatenates per-core outputs.

Kernel structure per core (everything stays on-chip after one 4MB x load):
  1. GroupNorm via bn_stats + two tiny cross-partition matmul reductions.
  2. Q^T/K^T computed directly transposed ([d=64 part, s free]) so the score
     matmul needs no transposes; V computed in [s part, d free] with an
     appended ones-column.
  3. Per (head, 1024-query chunk): loop over 32 key blocks of 128:
     scores^T block [sk=128, sq=1024] on PE -> exp(0.125*x) on ScalarE (no
     max-subtraction: scores are provably O(+-8) here) -> PV matmul
     accumulating [65, 1024] in PSUM, where row 64 accumulates the softmax
     denominator (ones column of V).
  4. Normalize by the denominator (DMA partition-broadcast of the reciprocal
     row), out-projection accumulated over heads in PSUM, + bias + residual.
"""

import os
import numpy as np

import concourse.bass as bass
import concourse.tile as tile
from concourse import bacc, mybir
from concourse.bass_utils import run_bass_kernel_spmd

B, C, H, W = 4, 256, 64, 64
S = H * W            # 4096 spatial positions
NG = 32              # groupnorm groups
GS = C // NG         # 8 channels / group
NH, DK = 4, 64       # heads, head dim
SQ = S // 2          # queries per core (2048)
P = 128              # partitions
KC = C // P          # channel chunks (2)
EPS = 1e-5
SCALE = DK ** -0.5   # 0.125
N_CORES = 8

F32 = mybir.dt.float32
BF16 = mybir.dt.bfloat16
AL = mybir.AluOpType


def _build_nc():
    nc = bacc.Bacc("TRN2", target_bir_lowering=False, debug=False)

    d_x = nc.dram_tensor("x", [C, S], F32, kind="ExternalInput")
    d_gamma = nc.dram_tensor("gamma", [C], F32, kind="ExternalInput")
    d_beta = nc.dram_tensor("beta", [C], F32, kind="ExternalInput")
    d_wqkv = nc.dram_tensor("w_qkv", [C, 3 * NH * DK], F32, kind="ExternalInput")
    d_bqkv = nc.dram_tensor("b_qkv", [3 * NH * DK], F32, kind="ExternalInput")
    d_wout = nc.dram_tensor("w_out", [NH * DK, C], F32, kind="ExternalInput")
    d_bout = nc.dram_tensor("b_out", [C], F32, kind="ExternalInput")
    # group-average / group-broadcast one-hot matrices (host-precomputed)
    d_c2g = nc.dram_tensor("c2g", [C, NG], F32, kind="ExternalInput")
    d_g2c = nc.dram_tensor("g2c", [NG, C], F32, kind="ExternalInput")
    d_out = nc.dram_tensor("out", [C, SQ], F32, kind="ExternalOutput")

    dbg = {}
    if os.environ.get("KDBG"):
        dbg["h"] = nc.dram_tensor("dbg_h", [P, KC, S], BF16, kind="ExternalOutput")
        dbg["qt"] = nc.dram_tensor("dbg_qt", [DK, NH, SQ], BF16, kind="ExternalOutput")
        dbg["kt"] = nc.dram_tensor("dbg_kt", [DK, NH, S], BF16, kind="ExternalOutput")
        dbg["v"] = nc.dram_tensor("dbg_v", [P, S // P, NH, DK + 1], BF16,
                                  kind="ExternalOutput")
        dbg["at"] = nc.dram_tensor("dbg_at", [DK, NH, SQ], BF16, kind="ExternalOutput")

    with tile.TileContext(nc) as tc:
        _body(tc, d_x, d_gamma, d_beta, d_wqkv, d_bqkv, d_wout, d_bout,
              d_c2g, d_g2c, d_out, dbg)

    nc.compile()
    return nc


def _body(tc, d_x, d_gamma, d_beta, d_wqkv, d_bqkv, d_wout, d_bout,
          d_c2g, d_g2c, d_out, dbg=None):
    dbg = dbg or {}
    nc = tc.nc
    from contextlib import ExitStack

    with ExitStack() as ctx:
        consts = ctx.enter_context(tc.tile_pool(name="consts", bufs=1))
        big = ctx.enter_context(tc.tile_pool(name="big", bufs=1))
        work = ctx.enter_context(tc.tile_pool(name="work", bufs=2))

        # ------------- loads -------------
        x_sb = big.tile([P, KC, S], F32, name="x_sb")
        nc.sync.dma_start(out=x_sb, in_=d_x.ap().rearrange("(k p) s -> p k s", p=P))

        c2g = consts.tile([P, KC, NG], F32, name="c2g_sb")
        nc.sync.dma_start(out=c2g, in_=d_c2g.ap().rearrange("(k p) g -> p k g", p=P))
        g2c = consts.tile([NG, C], F32, name="g2c_sb")
        nc.sync.dma_start(out=g2c, in_=d_g2c.ap())
        gam = consts.tile([P, KC], F32, name="gam_sb")
        nc.sync.dma_start(out=gam, in_=d_gamma.ap().rearrange("(k p) -> p k", p=P))
        bet = consts.tile([P, KC], F32, name="bet_sb")
        nc.sync.dma_start(out=bet, in_=d_beta.ap().rearrange("(k p) -> p k", p=P))
        bout = consts.tile([P, KC], F32, name="bout_sb")
        nc.sync.dma_start(out=bout, in_=d_bout.ap().rearrange("(k p) -> p k", p=P))

        wqkv_f = big.tile([P, KC, 3 * NH * DK], F32, name="wqkv_f")
        nc.sync.dma_start(out=wqkv_f,
                          in_=d_wqkv.ap().rearrange("(k p) n -> p k n", p=P))
        wqkv = big.tile([P, KC, 3 * NH * DK], BF16, name="wqkv_sb")
        nc.vector.tensor_copy(out=wqkv, in_=wqkv_f)

        wout_f = big.tile([DK, NH, C], F32, name="wout_f")
        nc.sync.dma_start(out=wout_f,
                          in_=d_wout.ap().rearrange("(h d) c -> d h c", h=NH))
        wout = big.tile([DK, NH, C], BF16, name="wout_sb")
        nc.vector.tensor_copy(out=wout, in_=wout_f)

        # qkv bias: b_qkv layout is (head, {q,k,v}, d); load as [64, h*3+t]
        bqkv = consts.tile([DK, 3 * NH], F32, name="bqkv_sb")
        nc.sync.dma_start(
            out=bqkv,
            in_=d_bqkv.ap().rearrange("(x d) -> d x", d=DK))
        bq = bqkv.rearrange("d (h t) -> d t h", t=3)[:, 0, :]
        bk = bqkv.rearrange("d (h t) -> d t h", t=3)[:, 1, :]
        bv = bqkv.rearrange("d (h t) -> d t h", t=3)[:, 2, :]

        eps_t = consts.tile([NG, 1], F32, name="eps_t")
        nc.vector.memset(eps_t, EPS)

        # ------------- GroupNorm -------------
        h_sb = big.tile([P, KC, S], BF16, name="h_sb")
        with tc.tile_pool(name="psA", bufs=2, space="PSUM") as psA:
            gp = psA.tile([NG, 3], F32, tag="gp", bufs=1)
            for k in range(KC):
                st = work.tile([P, 8, 6], F32, tag="bnst")
                for sub in range(8):
                    nc.vector.bn_stats(out=st[:, sub, :],
                                       in_=x_sb[:, k, sub * 512:(sub + 1) * 512])
                mv = work.tile([P, 2], F32, tag="mv")
                nc.vector.bn_aggr(out=mv, in_=st)
                s3 = work.tile([P, 3], F32, tag="s3")
                nc.vector.tensor_copy(out=s3[:, 0:2], in_=mv)
                nc.vector.tensor_mul(out=s3[:, 2:3], in0=mv[:, 0:1], in1=mv[:, 0:1])
                # group stats: [32, 3] = (mean, E[var], E[mean^2]) averaged over
                # the 8 channels of each group (c2g holds 1/8)
                nc.tensor.matmul(out=gp, lhsT=c2g[:, k, :], rhs=s3,
                                 start=(k == 0), stop=(k == KC - 1))
            gsb = work.tile([NG, 3], F32, tag="gsb")
            nc.vector.tensor_copy(out=gsb, in_=gp)
            varg = work.tile([NG, 1], F32, tag="varg")
            nc.vector.tensor_add(out=varg, in0=gsb[:, 1:2], in1=gsb[:, 2:3])
            m2 = work.tile([NG, 1], F32, tag="m2")
            nc.vector.tensor_mul(out=m2, in0=gsb[:, 0:1], in1=gsb[:, 0:1])
            nc.vector.tensor_sub(out=varg, in0=varg, in1=m2)
            stdg = work.tile([NG, 1], F32, tag="stdg")
            nc.scalar.activation(out=stdg, in_=varg,
                                 func=mybir.ActivationFunctionType.Sqrt,
                                 bias=eps_t, scale=1.0)
            rstdg = work.tile([NG, 1], F32, tag="rstdg")
            nc.vector.reciprocal(out=rstdg, in_=stdg)
            gs2 = work.tile([NG, 2], F32, tag="gs2")
            nc.vector.tensor_copy(out=gs2[:, 0:1], in_=gsb[:, 0:1])
            nc.vector.tensor_copy(out=gs2[:, 1:2], in_=rstdg)

            for k in range(KC):
                cp = psA.tile([P, 2], F32, tag="cp")
                nc.tensor.matmul(out=cp, lhsT=g2c[:, k * P:(k + 1) * P], rhs=gs2,
                                 start=True, stop=True)
                scale_c = work.tile([P, 1], F32, tag="scale_c")
                nc.vector.tensor_mul(out=scale_c, in0=gam[:, k:k + 1],
                                     in1=cp[:, 1:2])
                tmpc = work.tile([P, 1], F32, tag="tmpc")
                nc.vector.tensor_mul(out=tmpc, in0=cp[:, 0:1], in1=scale_c)
                bias_c = work.tile([P, 1], F32, tag="bias_c")
                nc.vector.tensor_sub(out=bias_c, in0=bet[:, k:k + 1], in1=tmpc)
                nc.vector.tensor_scalar(out=h_sb[:, k, :], in0=x_sb[:, k, :],
                                        scalar1=scale_c, scalar2=bias_c,
                                        op0=AL.mult, op1=AL.add)

        if "h" in dbg:
            nc.sync.dma_start(out=dbg["h"].ap(), in_=h_sb)

        # ------------- QKV projections -------------
        QT = big.tile([DK, NH, SQ], BF16, name="QT")
        KT = big.tile([DK, NH, S], BF16, name="KT")
        V = big.tile([P, S // P, NH, DK + 1], BF16, name="V")
        nc.vector.memset(V[:, :, :, DK:DK + 1], 1.0)

        with tc.tile_pool(name="psB", bufs=3, space="PSUM") as psB:
            for h in range(NH):
                for dst, colbase, srcS, btile in (
                        (KT, h * 3 * DK + DK, S, bk),
                        (QT, h * 3 * DK, SQ, bq)):
                    for stn in range(srcS // 512):
                        pq = psB.tile([DK, 512], F32, tag="qk")
                        for k in range(KC):
                            nc.tensor.matmul(
                                out=pq,
                                lhsT=wqkv[:, k, colbase:colbase + DK],
                                rhs=h_sb[:, k, stn * 512:(stn + 1) * 512],
                                start=(k == 0), stop=(k == KC - 1))
                        nc.vector.tensor_scalar(
                            out=dst[:, h, stn * 512:(stn + 1) * 512],
                            in0=pq, scalar1=btile[:, h:h + 1], scalar2=None,
                            op0=AL.add)
            for sc in range(S // P):
                pv = psB.tile([P, NH * DK], F32, tag="v")
                for k in range(KC):
                    wv_view = wqkv[:, k, :].rearrange(
                        "p (h t d) -> p t h d", h=NH, t=3)[:, 2, :, :]
                    nc.tensor.matmul(out=pv,
                                     lhsT=h_sb[:, k, sc * P:(sc + 1) * P],
                                     rhs=wv_view,
                                     start=(k == 0), stop=(k == KC - 1))
                nc.vector.tensor_copy(
                    out=V[:, sc, :, 0:DK],
                    in_=pv.rearrange("p (h d) -> p h d", h=NH))

        if "qt" in dbg:
            nc.sync.dma_start(out=dbg["qt"].ap(), in_=QT)
            nc.sync.dma_start(out=dbg["kt"].ap(), in_=KT)
            nc.sync.dma_start(out=dbg["v"].ap(), in_=V)

        # ------------- attention -------------
        aT = big.tile([DK, NH, SQ], BF16, name="aT")
        CH = 1024  # query chunk
        with tc.tile_pool(name="psC", bufs=2, space="PSUM") as psC, \
                tc.tile_pool(name="ptp", bufs=3) as ptp, \
                tc.tile_pool(name="rbp", bufs=2) as rbp:
            NJ = S // P

            def do_scores(h, q0, j):
                # scores^T block [sk=128, sq=CH] for key block j
                sc_ps = psC.tile([P, CH], F32, tag="sc", name=f"sc_{h}_{q0}_{j}")
                for hf in range(CH // 512):
                    nc.tensor.matmul(
                        out=sc_ps[:, hf * 512:(hf + 1) * 512],
                        lhsT=KT[:, h, j * P:(j + 1) * P],
                        rhs=QT[:, h, q0 + hf * 512:q0 + (hf + 1) * 512],
                        start=True, stop=True)
                return sc_ps

            def do_exp(sc_ps, h, q0, j):
                pT = ptp.tile([P, CH], BF16, tag="pt", name=f"pt_{h}_{q0}_{j}")
                nc.scalar.activation(out=pT, in_=sc_ps,
                                     func=mybir.ActivationFunctionType.Exp,
                                     scale=SCALE)
                return pT

            def do_pv(pvacc, pT, h, j):
                for hf in range(CH // 512):
                    nc.tensor.matmul(
                        out=pvacc[:, hf * 512:(hf + 1) * 512],
                        lhsT=V[:, j, h, :],
                        rhs=pT[:, hf * 512:(hf + 1) * 512],
                        start=(j == 0), stop=(j == NJ - 1))

            for h in range(NH):
                for chi in range(SQ // CH):
                    q0 = chi * CH
                    pvacc = psC.tile([DK + 1, CH], F32, tag="pv",
                                     name=f"pv_{h}_{chi}")
                    # software pipeline: PE program order is
                    # sc_0, sc_1, [pv_0, sc_2], [pv_1, sc_3], ... so the PE
                    # never sits directly behind the exp of the block it is
                    # about to consume.
                    sc_prev = do_scores(h, q0, 0)
                    pT_prev = do_exp(sc_prev, h, q0, 0)
                    for j in range(1, NJ):
                        sc_cur = do_scores(h, q0, j)
                        do_pv(pvacc, pT_prev, h, j - 1)
                        pT_prev = do_exp(sc_cur, h, q0, j)
                    do_pv(pvacc, pT_prev, h, NJ - 1)
                    # normalize: aT = pvacc[0:64] * (1 / l) + b_v
                    rec = rbp.tile([1, CH], F32, tag="rec")
                    nc.vector.reciprocal(out=rec, in_=pvacc[DK:DK + 1, :])
                    rb = rbp.tile([DK, CH], F32, tag="rb")
                    nc.gpsimd.partition_broadcast(rb, rec)
                    nc.vector.tensor_mul(out=aT[:, h, q0:q0 + CH],
                                         in0=pvacc[0:DK, :], in1=rb)
                    nc.vector.tensor_scalar(out=aT[:, h, q0:q0 + CH],
                                            in0=aT[:, h, q0:q0 + CH],
                                            scalar1=bv[:, h:h + 1], scalar2=None,
                                            op0=AL.add)

        if "at" in dbg:
            nc.sync.dma_start(out=dbg["at"].ap(), in_=aT)

        # ------------- out projection + residual -------------
        out_view = d_out.ap().rearrange("(k p) s -> p k s", p=P)
        with tc.tile_pool(name="psD", bufs=2, space="PSUM") as psD, \
                tc.tile_pool(name="ost", bufs=3) as ost:
            for k in range(KC):
                for stn in range(SQ // 512):
                    po = psD.tile([P, 512], F32, tag="op")
                    for h in range(NH):
                        nc.tensor.matmul(out=po,
                                         lhsT=wout[:, h, k * P:(k + 1) * P],
                                         rhs=aT[:, h, stn * 512:(stn + 1) * 512],
                                         start=(h == 0), stop=(h == NH - 1))
                    ob = ost.tile([P, 512], F32, tag="ob")
                    nc.vector.tensor_scalar(out=ob, in0=po,
                                            scalar1=bout[:, k:k + 1], scalar2=None,
                                            op0=AL.add)
                    nc.vector.tensor_add(out=ob, in0=ob,
                                         in1=x_sb[:, k, stn * 512:(stn + 1) * 512])
                    nc.sync.dma_start(out=out_view[:, k, stn * 512:(stn + 1) * 512],
                                      in_=ob)


_NC_CACHE = {}


def _get_nc():
    if "nc" not in _NC_CACHE:
        _NC_CACHE["nc"] = _build_nc()
    return _NC_CACHE["nc"]


def _host_constants():
    c2g = np.zeros((C, NG), np.float32)
    c2g[np.arange(C), np.arange(C) // GS] = 1.0 / GS
    g2c = np.zeros((NG, C), np.float32)
    g2c[np.arange(C) // GS, np.arange(C)] = 1.0
    return c2g, g2c


def make_in_maps(x, gamma, beta, w_qkv, b_qkv, w_out, b_out):
    c2g, g2c = _host_constants()
    com = {
        "gamma": np.ascontiguousarray(np.asarray(gamma, np.float32)),
        "beta": np.ascontiguousarray(np.asarray(beta, np.float32)),
        "w_qkv": np.ascontiguousarray(np.asarray(w_qkv, np.float32)),
        "b_qkv": np.ascontiguousarray(np.asarray(b_qkv, np.float32)),
        "w_out": np.ascontiguousarray(np.asarray(w_out, np.float32)),
        "b_out": np.ascontiguousarray(np.asarray(b_out, np.float32)),
        "c2g": c2g, "g2c": g2c,
    }
    x = np.asarray(x, np.float32)
    in_maps = []
    for c in range(N_CORES):
        b, half = divmod(c, 2)
        xb = x[b].reshape(C, S)
        if half:
            xb = np.roll(xb, -half * SQ, axis=1)
        in_maps.append({"x": np.ascontiguousarray(xb), **com})
    return in_maps


def assemble(results):
    out = np.empty((B, C, S), np.float32)
    for c in range(N_CORES):
        b, half = divmod(c, 2)
        out[b, :, half * SQ:(half + 1) * SQ] = results[c]["out"]
    return out.reshape(B, C, H, W)


def kernel(x, gamma, beta, w_qkv, b_qkv, w_out, b_out):
    nc = _get_nc()
    in_maps = make_in_maps(x, gamma, beta, w_qkv, b_qkv, w_out, b_out)
    res = run_bass_kernel_spmd(nc, in_maps, list(range(N_CORES)))
    return assemble(res.results)
